# revision 31
# baseline (speedup 1.0000x reference)
"""Trainium2 Bass kernel for nn_CnUpdateLayer (segment_reduce / LDPC check-node update).

reference: out[b, i] = prod_{j : mask[i,j]==1} x[b, j]   (x ~ N(0,1), never exactly 0)

Log-domain trick turns the masked product into one dense matmul pass:
    S[b,i] = sum_j mask[i,j] * ln(x[b,j]^2)        -> magnitude = exp(0.5*S)
    C[b,i] = sum_j mask[i,j] * (x[b,j] < 0)        -> sign via parity of C
Both contractions share the stationary mask operand, so each K-tile runs as ONE
TensorEngine matmul with the moving operand [ln(x^2)^T | neg^T] (N=256,
float32r: full-rate fp32 matmul, fp32 PSUM accumulate).

Sharding: OUT columns (mask rows) x 8 cores, no collectives. Each core gets
x^T (replicated) and its fp8 mask shard (0/1 exact) pre-packed on host into
the exact SBUF image. Perf structure:
  - one HWDGE descriptor walk tops out ~170 GB/s, so x is split across the
    two HWDGE engines (sync + scalar) for parallel walks
  - fp8 mask lands first and is upcast to f32r on DVE before x arrives
  - DVE/ACT/PE pipeline in two K-chunks ordered by DMA arrival
  - epilogue: DVE reads the C count from PSUM first, then ACT's exp runs
    concurrently with DVE's shift (PSUM bank reads must never overlap
    across engines - that faults the exec unit)
  - no Tile framework, no end-of-kernel barrier: hand-placed semaphores,
    every DMA completion explicitly waited

Hardcoded problem shape: x [128, 1024] f32, layer_mask [1024, 1024] f32.
"""

import sys
from contextlib import ExitStack

import numpy as np

sys.path.insert(0, "/opt/trn_rl_repo")

import concourse.bacc as bacc
import concourse.bass as bass
from concourse import mybir
from concourse.bass_utils import run_bass_kernel_spmd

B = 128          # batch
IN = 1024        # in_features (contraction dim)
OUT = 1024       # out_features
NCORES = 8
O_SHARD = OUT // NCORES   # 128 mask rows per core
NK = IN // 128            # 8 K-tiles
KPC = NK // 2             # K-tiles per compute chunk

F32 = mybir.dt.float32
F32R = mybir.dt.float32r
F8 = mybir.dt.float8e4
I32 = mybir.dt.int32
AF = mybir.ActivationFunctionType
ALU = mybir.AluOpType

# act_func_sets[6] = natural_log_exp_and_others: serves ln + exp (+ square/copy)
ACT_TABLE_LN_EXP = 6


def build_nc():
    nc = bacc.Bacc(None, target_bir_lowering=False)
    # Host pre-packs the exact SBUF image: row p holds [aT[k*128+p, :] for k in NK]
    xt = nc.declare_dram_parameter("xt", [128, NK * B], F32, isOutput=False)
    mt = nc.declare_dram_parameter("mt", [128, NK * O_SHARD], F8, isOutput=False)
    out = nc.declare_dram_parameter("out", [O_SHARD, B], F32, isOutput=True)

    xt3 = xt[:, :].rearrange("p (k b) -> p k b", k=NK)
    mt3 = mt[:, :].rearrange("p (k o) -> p k o", k=NK)

    lo = slice(0, KPC)        # k 0..3  (arrives second, on sync after mask)
    hi = slice(KPC, NK)       # k 4..7  (arrives first, on scalar)

    with ExitStack() as ctx:
        xs = ctx.enter_context(nc.sbuf_tensor([128, NK, B], F32))
        m8 = ctx.enter_context(nc.sbuf_tensor([128, NK, O_SHARD], F8))
        ms = ctx.enter_context(nc.sbuf_tensor([128, NK, O_SHARD], F32R))
        sq = ctx.enter_context(nc.sbuf_tensor([128, NK, B], F32))
        ln = ctx.enter_context(nc.sbuf_tensor([128, NK, 2 * B], F32R))
        ps = ctx.enter_context(nc.psum_tensor([128, 2 * B], F32))
        mag = ctx.enter_context(nc.sbuf_tensor([128, B], F32))
        ci = ctx.enter_context(nc.sbuf_tensor([128, B], I32))
        res = ctx.enter_context(nc.sbuf_tensor([128, B], F32))

        dma_m = ctx.enter_context(nc.semaphore("dma_m"))
        dma_xl = ctx.enter_context(nc.semaphore("dma_xl"))
        dma_xh = ctx.enter_context(nc.semaphore("dma_xh"))
        dma_o = ctx.enter_context(nc.semaphore("dma_o"))
        s_m = ctx.enter_context(nc.semaphore("s_m"))
        s_sq = ctx.enter_context(nc.semaphore("s_sq"))
        s_ln = ctx.enter_context(nc.semaphore("s_ln"))
        s_neg = ctx.enter_context(nc.semaphore("s_neg"))
        s_pe = ctx.enter_context(nc.semaphore("s_pe"))
        s_mag = ctx.enter_context(nc.semaphore("s_mag"))
        s_epi = ctx.enter_context(nc.semaphore("s_epi"))

        block = bass.BassBlock(nc, f"block_{nc.next_id()}")
        nc.cur_block = block

        @block.sync
        def _(sync):
            # x low half on sync's HWDGE ring
            sync.dma_start(out=xs[:, lo, :], in_=xt3[:, lo, :]).then_inc(dma_xl, 16)
            # result -> DRAM, rows 0:64 (parallel with scalar's half)
            sync.wait_ge(s_epi, 3)
            sync.dma_start(out=out[0:64, :], in_=res[0:64, :]).then_inc(dma_o, 16)
            sync.wait_ge(dma_o, 32)

        @block.gpsimd
        def _(gpsimd):
            # fp8 mask via gpsimd's SWDGE ring (third parallel walker),
            # then upcast to f32r here - off DVE's critical path
            gpsimd.dma_start(out=m8[:, :, :], in_=mt3).then_inc(dma_m, 16)
            gpsimd.wait_ge(dma_m, 16)
            nc.gpsimd.tensor_copy(ms[:, :, :], m8[:, :, :]).then_inc(s_m, 1)

        @block.scalar
        def _(scalar):
            # x high half on the second HWDGE ring - parallel descriptor walk
            scalar.dma_start(out=xs[:, hi, :], in_=xt3[:, hi, :]).then_inc(dma_xh, 16)
            scalar.add_instruction(
                mybir.InstLoadActFuncSet(
                    name=nc.get_next_instruction_name(),
                    ins=[],
                    outs=[],
                    act_func_set_id=ACT_TABLE_LN_EXP,
                )
            )
            # ln over chunks in arrival order: hi first
            scalar.wait_ge(s_sq, 1)
            nc.scalar.activation(ln[:, hi, 0:B], sq[:, hi, :], AF.Ln).then_inc(s_ln, 1)
            scalar.wait_ge(s_sq, 2)
            nc.scalar.activation(ln[:, lo, 0:B], sq[:, lo, :], AF.Ln).then_inc(s_ln, 1)
            # epilogue magnitude: wait for DVE's C read of the PSUM bank
            scalar.wait_ge(s_epi, 1)
            nc.scalar.activation(mag[:, :], ps[:, 0:B], AF.Exp, scale=0.5).then_inc(
                s_mag, 1
            )
            # second half of the output from scalar's ring
            scalar.wait_ge(s_epi, 3)
            scalar.dma_start(out=out[64:128, :], in_=res[64:128, :]).then_inc(dma_o, 16)

        @block.vector
        def _(vector):
            # hi chunk (lands first)
            vector.wait_ge(dma_xh, 16)
            nc.vector.tensor_mul(
                sq[:, hi, :], xs[:, hi, :], xs[:, hi, :]
            ).then_inc(s_sq, 1)
            nc.vector.tensor_scalar(
                ln[:, hi, B:2 * B], xs[:, hi, :], 0.0, None, op0=ALU.is_lt
            ).then_inc(s_neg, 1)
            # lo chunk
            vector.wait_ge(dma_xl, 16)
            nc.vector.tensor_mul(
                sq[:, lo, :], xs[:, lo, :], xs[:, lo, :]
            ).then_inc(s_sq, 1)
            nc.vector.tensor_scalar(
                ln[:, lo, B:2 * B], xs[:, lo, :], 0.0, None, op0=ALU.is_lt
            ).then_inc(s_neg, 1)
            # epilogue: read C^T (exact-integer count) out of PSUM first so
            # ACT's exp can read the S half right after (never concurrently).
            vector.wait_ge(s_pe, 1)
            nc.vector.tensor_copy(ci[:, :], ps[:, B:2 * B]).then_inc(s_epi, 1)
            vector.wait_ge(s_epi, 1)
            nc.vector.tensor_scalar(
                ci[:, :], ci[:, :], 31, None, op0=ALU.logical_shift_left
            ).then_inc(s_epi, 1)
            # res = mag XOR ((C & 1) << 31): parity flips the float sign bit
            vector.wait_ge(s_epi, 2)
            vector.wait_ge(s_mag, 1)
            nc.vector.tensor_tensor(
                res[:, :].bitcast(I32), ci[:, :], mag[:, :].bitcast(I32),
                op=ALU.bitwise_xor,
            ).then_inc(s_epi, 1)

        @block.tensor
        def _(tensor):
            tensor.wait_ge(s_m, 1)
            first = True
            for chunk_i, ks in enumerate((hi, lo)):
                tensor.wait_ge(s_ln, chunk_i + 1)
                tensor.wait_ge(s_neg, chunk_i + 1)
                for k in range(ks.start, ks.stop):
                    mm = nc.tensor.matmul(
                        ps[:, :],
                        lhsT=ms[:, k, :],
                        rhs=ln[:, k, :],
                        start=first,
                        stop=(chunk_i == 1 and k == ks.stop - 1),
                    )
                    first = False
            mm.then_inc(s_pe, 1)

        nc.cur_block = None

    nc.finalize()
    return nc


_NC_CACHE = None


def _get_nc():
    global _NC_CACHE
    if _NC_CACHE is None:
        _NC_CACHE = build_nc()
    return _NC_CACHE


def _pack(aT: np.ndarray) -> np.ndarray:
    # [IN, W] -> [128, NK*W] SBUF image: row p = concat_k aT[k*128+p, :]
    w = aT.shape[1]
    return np.ascontiguousarray(
        aT.reshape(NK, 128, w).transpose(1, 0, 2).reshape(128, NK * w)
    )


def make_in_maps(x: np.ndarray, layer_mask: np.ndarray):
    f8np = mybir.dt.np(F8)
    xt = _pack(x.T.astype(np.float32))  # [128, NK*B]
    in_maps = []
    for c in range(NCORES):
        mt = _pack(
            layer_mask[c * O_SHARD:(c + 1) * O_SHARD, :].T.astype(np.float32)
        ).astype(f8np)  # [128, NK*O_SHARD] fp8 (0/1 exact)
        in_maps.append({"xt": xt, "mt": mt})
    return in_maps


def assemble_out(results):
    # results[c]["out"] is [O_SHARD, B] = out_full[:, shard].T
    return np.concatenate([r["out"].T for r in results], axis=1)


def run(x, layer_mask, trace=False, **kw):
    nc = _get_nc()
    in_maps = make_in_maps(np.asarray(x), np.asarray(layer_mask))
    res = run_bass_kernel_spmd(nc, in_maps, core_ids=list(range(NCORES)), trace=trace, **kw)
    return assemble_out(res.results), res


def kernel(x: np.ndarray, layer_mask: np.ndarray) -> np.ndarray:
    out, _ = run(x, layer_mask, trace=False)
    return out.astype(np.float32)


# revision 39
# speedup vs baseline: 1.1670x; 1.1670x over previous
"""Trainium2 Bass kernel for nn_CnUpdateLayer (segment_reduce / LDPC check-node update).

reference: out[b, i] = prod_{j : mask[i,j]==1} x[b, j]   (x ~ N(0,1), never exactly 0)

Log-domain trick turns the masked product into one dense matmul pass:
    S[b,i] = sum_j mask[i,j] * ln(x[b,j]^2)        -> magnitude = exp(0.5*S)
    C[b,i] = sum_j mask[i,j] * (x[b,j] < 0)        -> sign via parity of C
Both contractions share the stationary mask operand, so each K-tile runs as ONE
TensorEngine matmul with the moving operand [ln(x^2)^T | neg^T] (N=256,
float32r: full-rate fp32 matmul, fp32 PSUM accumulate).

Sharding: OUT columns (mask rows) x 8 cores, no collectives. Each core gets
x^T (replicated) and its fp8 mask shard (0/1 exact) pre-packed on host into
the exact SBUF image. Perf structure:
  - one HWDGE descriptor walk tops out ~170 GB/s, so x is split across the
    two HWDGE engines (sync + scalar) for parallel walks
  - fp8 mask lands first and is upcast to f32r on DVE before x arrives
  - DVE/ACT/PE pipeline in two K-chunks ordered by DMA arrival
  - epilogue: DVE reads the C count from PSUM first, then ACT's exp runs
    concurrently with DVE's shift (PSUM bank reads must never overlap
    across engines - that faults the exec unit)
  - no Tile framework, no end-of-kernel barrier: hand-placed semaphores,
    every DMA completion explicitly waited

Hardcoded problem shape: x [128, 1024] f32, layer_mask [1024, 1024] f32.
"""

import sys
from contextlib import ExitStack

import numpy as np

sys.path.insert(0, "/opt/trn_rl_repo")

import concourse.bacc as bacc
import concourse.bass as bass
from concourse import mybir
from concourse.bass_utils import run_bass_kernel_spmd

B = 128          # batch
IN = 1024        # in_features (contraction dim)
OUT = 1024       # out_features
NCORES = 8
O_SHARD = OUT // NCORES   # 128 mask rows per core
NK = IN // 128            # 8 K-tiles
KPC = NK // 2             # K-tiles per compute chunk

F32 = mybir.dt.float32
F32R = mybir.dt.float32r
F8 = mybir.dt.float8e4
I32 = mybir.dt.int32
AF = mybir.ActivationFunctionType
ALU = mybir.AluOpType

# act_func_sets[6] = natural_log_exp_and_others: serves ln + exp (+ square/copy)
ACT_TABLE_LN_EXP = 6


def build_nc():
    nc = bacc.Bacc(None, target_bir_lowering=False)
    # Host pre-packs the exact SBUF image: row p holds [aT[k*128+p, :] for k in NK]
    xt = nc.declare_dram_parameter("xt", [128, NK * B], F32, isOutput=False)
    mt = nc.declare_dram_parameter("mt", [128, NK * O_SHARD], F32R, isOutput=False)
    out = nc.declare_dram_parameter("out", [O_SHARD, B], F32, isOutput=True)

    xt3 = xt[:, :].rearrange("p (k b) -> p k b", k=NK)
    mt3 = mt[:, :].rearrange("p (k o) -> p k o", k=NK)

    lo = slice(0, KPC)        # k 0..3  (arrives second, on sync after mask)
    hi = slice(KPC, NK)       # k 4..7  (arrives first, on scalar)

    with ExitStack() as ctx:
        xs = ctx.enter_context(nc.sbuf_tensor([128, NK, B], F32))
        ms = ctx.enter_context(nc.sbuf_tensor([128, NK, O_SHARD], F32R))
        sq = ctx.enter_context(nc.sbuf_tensor([128, NK, B], F32))
        ln = ctx.enter_context(nc.sbuf_tensor([128, NK, 2 * B], F32R))
        ps = ctx.enter_context(nc.psum_tensor([128, 2 * B], F32))
        mag = ctx.enter_context(nc.sbuf_tensor([128, B], F32))
        ci = ctx.enter_context(nc.sbuf_tensor([128, B], I32))
        res = ctx.enter_context(nc.sbuf_tensor([128, B], F32))

        dma_ml = ctx.enter_context(nc.semaphore("dma_ml"))
        dma_mh = ctx.enter_context(nc.semaphore("dma_mh"))
        dma_xl = ctx.enter_context(nc.semaphore("dma_xl"))
        dma_xh = ctx.enter_context(nc.semaphore("dma_xh"))
        dma_o = ctx.enter_context(nc.semaphore("dma_o"))
        s_sq = ctx.enter_context(nc.semaphore("s_sq"))
        s_ln = ctx.enter_context(nc.semaphore("s_ln"))
        s_neg = ctx.enter_context(nc.semaphore("s_neg"))
        s_pe = ctx.enter_context(nc.semaphore("s_pe"))
        s_mag = ctx.enter_context(nc.semaphore("s_mag"))
        s_epi = ctx.enter_context(nc.semaphore("s_epi"))

        block = bass.BassBlock(nc, f"block_{nc.next_id()}")
        nc.cur_block = block

        @block.sync
        def _(sync):
            # x low half, then mask low half, on sync's HWDGE ring
            sync.dma_start(out=xs[:, lo, :], in_=xt3[:, lo, :]).then_inc(dma_xl, 16)
            sync.dma_start(out=ms[:, lo, :], in_=mt3[:, lo, :]).then_inc(dma_ml, 16)
            # result -> DRAM, rows 0:64 (parallel with scalar's half)
            sync.wait_ge(s_epi, 3)
            sync.dma_start(out=out[0:64, :], in_=res[0:64, :]).then_inc(dma_o, 16)
            sync.wait_ge(dma_o, 32)

        @block.scalar
        def _(scalar):
            # x high half + mask high half on the second HWDGE ring
            scalar.dma_start(out=xs[:, hi, :], in_=xt3[:, hi, :]).then_inc(dma_xh, 16)
            scalar.dma_start(out=ms[:, hi, :], in_=mt3[:, hi, :]).then_inc(dma_mh, 16)
            scalar.add_instruction(
                mybir.InstLoadActFuncSet(
                    name=nc.get_next_instruction_name(),
                    ins=[],
                    outs=[],
                    act_func_set_id=ACT_TABLE_LN_EXP,
                )
            )
            # ln over chunks in arrival order: hi first
            scalar.wait_ge(s_sq, 1)
            nc.scalar.activation(ln[:, hi, 0:B], sq[:, hi, :], AF.Ln).then_inc(s_ln, 1)
            scalar.wait_ge(s_sq, 2)
            nc.scalar.activation(ln[:, lo, 0:B], sq[:, lo, :], AF.Ln).then_inc(s_ln, 1)
            # epilogue magnitude: wait for DVE's C read of the PSUM bank
            scalar.wait_ge(s_epi, 1)
            nc.scalar.activation(mag[:, :], ps[:, 0:B], AF.Exp, scale=0.5).then_inc(
                s_mag, 1
            )
            # second half of the output from scalar's ring
            scalar.wait_ge(s_epi, 3)
            scalar.dma_start(out=out[64:128, :], in_=res[64:128, :]).then_inc(dma_o, 16)

        @block.vector
        def _(vector):
            # hi chunk (lands first)
            vector.wait_ge(dma_xh, 16)
            nc.vector.tensor_mul(
                sq[:, hi, :], xs[:, hi, :], xs[:, hi, :]
            ).then_inc(s_sq, 1)
            nc.vector.tensor_scalar(
                ln[:, hi, B:2 * B], xs[:, hi, :], 0.0, None, op0=ALU.is_lt
            ).then_inc(s_neg, 1)
            # lo chunk
            vector.wait_ge(dma_xl, 16)
            nc.vector.tensor_mul(
                sq[:, lo, :], xs[:, lo, :], xs[:, lo, :]
            ).then_inc(s_sq, 1)
            nc.vector.tensor_scalar(
                ln[:, lo, B:2 * B], xs[:, lo, :], 0.0, None, op0=ALU.is_lt
            ).then_inc(s_neg, 1)
            # epilogue: read C^T (exact-integer count) out of PSUM first so
            # ACT's exp can read the S half right after (never concurrently).
            vector.wait_ge(s_pe, 1)
            nc.vector.tensor_copy(ci[:, :], ps[:, B:2 * B]).then_inc(s_epi, 1)
            vector.wait_ge(s_epi, 1)
            nc.vector.tensor_scalar(
                ci[:, :], ci[:, :], 31, None, op0=ALU.logical_shift_left
            ).then_inc(s_epi, 1)
            # res = mag XOR ((C & 1) << 31): parity flips the float sign bit
            vector.wait_ge(s_epi, 2)
            vector.wait_ge(s_mag, 1)
            nc.vector.tensor_tensor(
                res[:, :].bitcast(I32), ci[:, :], mag[:, :].bitcast(I32),
                op=ALU.bitwise_xor,
            ).then_inc(s_epi, 1)

        @block.tensor
        def _(tensor):
            first = True
            for chunk_i, ks in enumerate((hi, lo)):
                tensor.wait_ge(dma_mh if chunk_i == 0 else dma_ml, 16)
                tensor.wait_ge(s_ln, chunk_i + 1)
                tensor.wait_ge(s_neg, chunk_i + 1)
                for k in range(ks.start, ks.stop):
                    mm = nc.tensor.matmul(
                        ps[:, :],
                        lhsT=ms[:, k, :],
                        rhs=ln[:, k, :],
                        start=first,
                        stop=(chunk_i == 1 and k == ks.stop - 1),
                    )
                    first = False
            mm.then_inc(s_pe, 1)

        nc.cur_block = None

    nc.finalize()
    return nc


_NC_CACHE = None


def _get_nc():
    global _NC_CACHE
    if _NC_CACHE is None:
        _NC_CACHE = build_nc()
    return _NC_CACHE


def _pack(aT: np.ndarray) -> np.ndarray:
    # [IN, W] -> [128, NK*W] SBUF image: row p = concat_k aT[k*128+p, :]
    w = aT.shape[1]
    return np.ascontiguousarray(
        aT.reshape(NK, 128, w).transpose(1, 0, 2).reshape(128, NK * w)
    )


def make_in_maps(x: np.ndarray, layer_mask: np.ndarray):
    xt = _pack(x.T.astype(np.float32))  # [128, NK*B]
    in_maps = []
    for c in range(NCORES):
        mt = _pack(
            layer_mask[c * O_SHARD:(c + 1) * O_SHARD, :].T.astype(np.float32)
        )  # [128, NK*O_SHARD]
        in_maps.append({"xt": xt, "mt": mt})
    return in_maps


def assemble_out(results):
    # results[c]["out"] is [O_SHARD, B] = out_full[:, shard].T
    return np.concatenate([r["out"].T for r in results], axis=1)


def run(x, layer_mask, trace=False, **kw):
    nc = _get_nc()
    in_maps = make_in_maps(np.asarray(x), np.asarray(layer_mask))
    res = run_bass_kernel_spmd(nc, in_maps, core_ids=list(range(NCORES)), trace=trace, **kw)
    return assemble_out(res.results), res


def kernel(x: np.ndarray, layer_mask: np.ndarray) -> np.ndarray:
    out, _ = run(x, layer_mask, trace=False)
    return out.astype(np.float32)


# revision 40
# speedup vs baseline: 1.2257x; 1.0503x over previous
"""Trainium2 Bass kernel for nn_CnUpdateLayer (segment_reduce / LDPC check-node update).

reference: out[b, i] = prod_{j : mask[i,j]==1} x[b, j]   (x ~ N(0,1), never exactly 0)

Log-domain trick turns the masked product into one dense matmul pass:
    S[b,i] = sum_j mask[i,j] * ln|x[b,j]|          -> magnitude = exp(S)
    C[b,i] = sum_j mask[i,j] * (x[b,j] < 0)        -> sign via parity of C
Both contractions share the stationary mask operand, so each K-tile runs as ONE
TensorEngine matmul with the moving operand [ln|x|^T | neg^T] (N=256,
float32r: full-rate fp32 matmul, fp32 PSUM accumulate).

Sharding: OUT columns (mask rows) x 8 cores, no collectives. Each core gets
x^T (replicated) and its mask shard pre-packed on host into the exact SBUF
image (contraction dim on SBUF partitions, fully contiguous DMAs).

Perf structure (raw Bass, no Tile, no end barrier):
  - HWDGE descriptor walks are the DMA bottleneck and run serially per
    issuing engine, so traffic is split across both HWDGE engines
    (sync + scalar) in quarter-sized x/mask transfers, interleaved so the
    pipeline starts on the first quarter
  - |x| via a sign-bit mask on DVE (2x tensor_scalar mode), ln/exp on ACT
    from one preloaded activation table, negativity indicator on DVE
  - epilogue: DVE reads the C count from PSUM, then ACT's exp runs while
    DVE shifts (PSUM bank reads must never overlap across engines)
  - sign applied by XORing the parity into the float sign bit
  - output DMA split across both HWDGE rings

Hardcoded problem shape: x [128, 1024] f32, layer_mask [1024, 1024] f32.
"""

import sys
from contextlib import ExitStack

import numpy as np

sys.path.insert(0, "/opt/trn_rl_repo")

import concourse.bacc as bacc
import concourse.bass as bass
from concourse import mybir
from concourse.bass_utils import run_bass_kernel_spmd

B = 128          # batch
IN = 1024        # in_features (contraction dim)
OUT = 1024       # out_features
NCORES = 8
O_SHARD = OUT // NCORES   # 128 mask rows per core
NK = IN // 128            # 8 K-tiles

F32 = mybir.dt.float32
F32R = mybir.dt.float32r
I32 = mybir.dt.int32
AF = mybir.ActivationFunctionType
ALU = mybir.AluOpType

# act_func_sets[6] = natural_log_exp_and_others: serves ln + exp
ACT_TABLE_LN_EXP = 6

# quarters (2 K-tiles each) in processing order: sync ring carries q0,q1
# (k 0..3), scalar ring carries q2,q3 (k 4..7). Arrival order is
# q0 ~ q2 first, then q1 ~ q3; process q0,q2,q1,q3.
QUARTERS = [slice(0, 2), slice(4, 6), slice(2, 4), slice(6, 8)]


def build_nc():
    nc = bacc.Bacc(None, target_bir_lowering=False)
    # Host pre-packs the exact SBUF image: row p holds [aT[k*128+p, :] for k in NK]
    xt = nc.declare_dram_parameter("xt", [128, NK * B], F32, isOutput=False)
    mt = nc.declare_dram_parameter("mt", [128, NK * O_SHARD], F32R, isOutput=False)
    out = nc.declare_dram_parameter("out", [O_SHARD, B], F32, isOutput=True)

    xt3 = xt[:, :].rearrange("p (k b) -> p k b", k=NK)
    mt3 = mt[:, :].rearrange("p (k o) -> p k o", k=NK)

    with ExitStack() as ctx:
        xs = ctx.enter_context(nc.sbuf_tensor([128, NK, B], F32))
        ms = ctx.enter_context(nc.sbuf_tensor([128, NK, O_SHARD], F32R))
        ax = ctx.enter_context(nc.sbuf_tensor([128, NK, B], F32))
        ln = ctx.enter_context(nc.sbuf_tensor([128, NK, 2 * B], F32R))
        ps = ctx.enter_context(nc.psum_tensor([128, 2 * B], F32))
        mag = ctx.enter_context(nc.sbuf_tensor([128, B], F32))
        ci = ctx.enter_context(nc.sbuf_tensor([128, B], I32))
        res = ctx.enter_context(nc.sbuf_tensor([128, B], F32))

        dma_x = [ctx.enter_context(nc.semaphore(f"dma_x{q}")) for q in range(4)]
        dma_m = [ctx.enter_context(nc.semaphore(f"dma_m{q}")) for q in range(4)]
        dma_o = ctx.enter_context(nc.semaphore("dma_o"))
        s_abs = ctx.enter_context(nc.semaphore("s_abs"))
        s_ln = ctx.enter_context(nc.semaphore("s_ln"))
        s_neg = ctx.enter_context(nc.semaphore("s_neg"))
        s_pe = ctx.enter_context(nc.semaphore("s_pe"))
        s_mag = ctx.enter_context(nc.semaphore("s_mag"))
        s_epi = ctx.enter_context(nc.semaphore("s_epi"))

        block = bass.BassBlock(nc, f"block_{nc.next_id()}")
        nc.cur_block = block

        @block.sync
        def _(sync):
            # interleave x / mask quarters so the first compute quarter and
            # its mask land as early as possible (q0 = QUARTERS[0], q1 = [2])
            sync.dma_start(out=xs[:, QUARTERS[0], :], in_=xt3[:, QUARTERS[0], :]).then_inc(dma_x[0], 16)
            sync.dma_start(out=ms[:, QUARTERS[0], :], in_=mt3[:, QUARTERS[0], :]).then_inc(dma_m[0], 16)
            sync.dma_start(out=xs[:, QUARTERS[2], :], in_=xt3[:, QUARTERS[2], :]).then_inc(dma_x[2], 16)
            sync.dma_start(out=ms[:, QUARTERS[2], :], in_=mt3[:, QUARTERS[2], :]).then_inc(dma_m[2], 16)
            # result -> DRAM, rows 0:64 (parallel with scalar's half)
            sync.wait_ge(s_epi, 3)
            sync.dma_start(out=out[0:64, :], in_=res[0:64, :]).then_inc(dma_o, 16)
            sync.wait_ge(dma_o, 32)

        @block.scalar
        def _(scalar):
            scalar.dma_start(out=xs[:, QUARTERS[1], :], in_=xt3[:, QUARTERS[1], :]).then_inc(dma_x[1], 16)
            scalar.dma_start(out=ms[:, QUARTERS[1], :], in_=mt3[:, QUARTERS[1], :]).then_inc(dma_m[1], 16)
            scalar.dma_start(out=xs[:, QUARTERS[3], :], in_=xt3[:, QUARTERS[3], :]).then_inc(dma_x[3], 16)
            scalar.dma_start(out=ms[:, QUARTERS[3], :], in_=mt3[:, QUARTERS[3], :]).then_inc(dma_m[3], 16)
            scalar.add_instruction(
                mybir.InstLoadActFuncSet(
                    name=nc.get_next_instruction_name(),
                    ins=[],
                    outs=[],
                    act_func_set_id=ACT_TABLE_LN_EXP,
                )
            )
            for q, ks in enumerate(QUARTERS):
                scalar.wait_ge(s_abs, q + 1)
                nc.scalar.activation(ln[:, ks, 0:B], ax[:, ks, :], AF.Ln).then_inc(s_ln, 1)
            # epilogue magnitude: wait for DVE's C read of the PSUM bank
            scalar.wait_ge(s_epi, 1)
            nc.scalar.activation(mag[:, :], ps[:, 0:B], AF.Exp).then_inc(s_mag, 1)
            # second half of the output from scalar's ring
            scalar.wait_ge(s_epi, 3)
            scalar.dma_start(out=out[64:128, :], in_=res[64:128, :]).then_inc(dma_o, 16)

        @block.vector
        def _(vector):
            for q, ks in enumerate(QUARTERS):
                vector.wait_ge(dma_x[q], 16)
                # |x| by clearing the sign bit (2x tensor_scalar mode)
                nc.vector.tensor_scalar(
                    ax[:, ks, :].bitcast(I32), xs[:, ks, :].bitcast(I32),
                    0x7FFFFFFF, None, op0=ALU.bitwise_and,
                ).then_inc(s_abs, 1)
                # neg indicator (x < 0) -> 1.0 / 0.0
                nc.vector.tensor_scalar(
                    ln[:, ks, B:2 * B], xs[:, ks, :], 0.0, None, op0=ALU.is_lt
                ).then_inc(s_neg, 1)
            # epilogue: read C^T (exact-integer count) out of PSUM first so
            # ACT's exp can read the S half right after (never concurrently).
            vector.wait_ge(s_pe, 1)
            nc.vector.tensor_copy(ci[:, :], ps[:, B:2 * B]).then_inc(s_epi, 1)
            vector.wait_ge(s_epi, 1)
            nc.vector.tensor_scalar(
                ci[:, :], ci[:, :], 31, None, op0=ALU.logical_shift_left
            ).then_inc(s_epi, 1)
            # res = mag XOR ((C & 1) << 31): parity flips the float sign bit
            vector.wait_ge(s_epi, 2)
            vector.wait_ge(s_mag, 1)
            nc.vector.tensor_tensor(
                res[:, :].bitcast(I32), ci[:, :], mag[:, :].bitcast(I32),
                op=ALU.bitwise_xor,
            ).then_inc(s_epi, 1)

        @block.tensor
        def _(tensor):
            first = True
            for q, ks in enumerate(QUARTERS):
                tensor.wait_ge(dma_m[q], 16)
                tensor.wait_ge(s_ln, q + 1)
                tensor.wait_ge(s_neg, q + 1)
                for k in range(ks.start, ks.stop):
                    mm = nc.tensor.matmul(
                        ps[:, :],
                        lhsT=ms[:, k, :],
                        rhs=ln[:, k, :],
                        start=first,
                        stop=(q == 3 and k == ks.stop - 1),
                    )
                    first = False
            mm.then_inc(s_pe, 1)

        nc.cur_block = None

    nc.finalize()
    return nc


_NC_CACHE = None


def _get_nc():
    global _NC_CACHE
    if _NC_CACHE is None:
        _NC_CACHE = build_nc()
    return _NC_CACHE


def _pack(aT: np.ndarray) -> np.ndarray:
    # [IN, W] -> [128, NK*W] SBUF image: row p = concat_k aT[k*128+p, :]
    w = aT.shape[1]
    return np.ascontiguousarray(
        aT.reshape(NK, 128, w).transpose(1, 0, 2).reshape(128, NK * w)
    )


def make_in_maps(x: np.ndarray, layer_mask: np.ndarray):
    xt = _pack(x.T.astype(np.float32))  # [128, NK*B]
    in_maps = []
    for c in range(NCORES):
        mt = _pack(
            layer_mask[c * O_SHARD:(c + 1) * O_SHARD, :].T.astype(np.float32)
        )  # [128, NK*O_SHARD]
        in_maps.append({"xt": xt, "mt": mt})
    return in_maps


def assemble_out(results):
    # results[c]["out"] is [O_SHARD, B] = out_full[:, shard].T
    return np.concatenate([r["out"].T for r in results], axis=1)


def run(x, layer_mask, trace=False, **kw):
    nc = _get_nc()
    in_maps = make_in_maps(np.asarray(x), np.asarray(layer_mask))
    res = run_bass_kernel_spmd(nc, in_maps, core_ids=list(range(NCORES)), trace=trace, **kw)
    return assemble_out(res.results), res


def kernel(x: np.ndarray, layer_mask: np.ndarray) -> np.ndarray:
    out, _ = run(x, layer_mask, trace=False)
    return out.astype(np.float32)
